# revision 20
# baseline (speedup 1.0000x reference)
"""Trainium2 Bass kernel for nn_CorrelationFilter (SiamFC-style correlation).

Math (per batch pair b):
    out[b, oi, oj] = sum_{di<6, dj<6, c<256} x[b, oi+di, oj+dj, c] * z[b, di, dj, c]
                     + sum_{c<256} bias[0, oi, oj, b*256 + c]
with x: [B,22,22,256], z: [B,6,6,256], bias: [1,17,17,B*256], out: [B,17,17,1].

Strategy: pure data parallelism over batch across 8 NeuronCores (16 batches per
core), no cross-core communication. Host does sharding + layout prep (transpose
to channel-major, cast to bf16) and pre-reduces the bias over its channel axis
(mathematically exact: bias enters the output only via sum_c).

Per-core layouts (DM = 3 di's merged per block, NK = 2 blocks, G = 18 groups;
group order g = djH*9 + dd*3 + dja with dj = 3*djH + dja):
  xT [2,128,16,484]      : xT[ch,c,b,p] = x[b, p//22, p%22, ch*128+c]
  zT [2,NK,128,16,G]     : zT[ch,k,c,b,g] = z[b, 3k+dd, dj, ch*128+c]
  bsum [16,289] f32      : bsum[b,o] = sum_c bias[0, o//17, o%17, b*256+c]

Device, one PSUM bank per batch:
  - 4 matmuls (ch,k): stationary zT[ch,k,:,b,:] (K=128, M=18), moving
    xT[ch][:, b, 66k : 66k+418], accumulating:
      Q[g, m] = group (djH,dd,dja) partial at column shift 22dd + 3djH + dja
  - ScalarE evacuation [18, 418] PSUM -> SBUF with f32->bf16 cast
  - one SBUF->SBUF DMA per batch transposes to t16[b, g, m]
  - fold: the shifted group access is affine (addr = 3765 djH + 1276 dd
    + 419 dja + m), so each dj-half folds in ONE strided tensor_reduce:
    Vector takes djH=0, GpSimd takes djH=1, then merge + bias add.

kernel(**inputs) takes FULL unsharded inputs, returns the full output.
"""

import os
import numpy as np
import ml_dtypes

import concourse.bass as bass
import concourse.mybir as mybir
from concourse import bacc
from concourse.tile import TileContext

B, H, W, C = 128, 22, 22, 256
HZ, WZ = 6, 6
HO, WO = 17, 17
OO = HO * WO               # 289 dense output positions
NCORES = 8
BPC = B // NCORES          # 16 batches per core
P = H * W                  # 484 flattened search positions
O22 = (HO - 1) * W + WO    # 369: output span in 22-wide layout

DM = 3                     # di's merged per matmul block
NK = HZ // DM              # 2 matmul blocks per (ch)
G = DM * WZ                # 18 fold groups per batch
NMOV = O22 + (DM - 1) * W + (WZ - 1)  # 418 moving cols per matmul
GH = G // 2                # 9 groups per dj-half
HIW = NMOV - 3             # 415 cols kept after the dj-half prefold
MZ = 41                    # stationary cols: [djH0 9 | zeros | djH1 9 @ 32]

_BF16 = mybir.dt.bfloat16
_F32 = mybir.dt.float32


def build_module():
    nc = bacc.Bacc()
    xt_d = nc.dram_tensor("xt", [2, 128, BPC, P], _BF16, kind="ExternalInput")
    zt_d = nc.dram_tensor("zt", [2, NK, 128, BPC, MZ], _BF16, kind="ExternalInput")
    bs_d = nc.dram_tensor("bs", [BPC, OO], _BF16, kind="ExternalInput")
    out_d = nc.dram_tensor("out", [BPC, HO, WO], _F32, kind="ExternalOutput")

    with TileContext(nc) as tc:
        with (
            tc.tile_pool(name="const", bufs=1) as cpool,
            tc.tile_pool(name="big", bufs=1) as big,
            tc.tile_pool(name="evac", bufs=6) as epool,
            tc.tile_pool(name="work", bufs=1) as work,
            tc.tile_pool(name="psum", bufs=8, space="PSUM") as psum,
        ):
            # stationary z split across queues: ch0 gates the first matmul,
            # so it rides sync ahead of the xt stream (the scalar queue's
            # ACT_TABLE_LOAD preamble would delay it); ch1+bsum on gpsimd
            zt_t = cpool.tile([128, 2, NK, BPC, MZ], _BF16, name="ztt")
            nc.scalar.dma_start(
                out=zt_t[:, 0], in_=zt_d[0].rearrange("k c b g -> c k b g")
            )
            nc.gpsimd.dma_start(
                out=zt_t[:, 1], in_=zt_d[1].rearrange("k c b g -> c k b g")
            )
            bsum = cpool.tile([BPC, OO], _BF16, name="bsum")
            nc.gpsimd.dma_start(out=bsum[:], in_=bs_d[:])

            xt_t = [
                big.tile([128, BPC, P], _BF16, name=f"xt{ch}", tag=f"xt{ch}")
                for ch in range(2)
            ]
            # t16[b, j, m] = Q_b[j, m] + Q_b[hi j, m+3] (dj-halves prefolded
            # in PSUM by the evacuation add; rows j = dd*3 + dja)
            t16 = big.tile([BPC, GH, HIW], _BF16, name="t16")

            # xt chunk schedule: small first chunks to unblock matmul 0 early
            chunks = {0: 2, 2: 2, 4: 4, 8: 4, 12: 4}

            for b in range(BPC):
                if b in chunks:
                    n = chunks[b]
                    for ch in range(2):
                        nc.sync.dma_start(
                            out=xt_t[ch][:, b : b + n, :],
                            in_=xt_d[ch, :, b : b + n, :],
                        )
                q1 = psum.tile([MZ, NMOV], _F32, name="q1", tag="q1", bufs=8)
                mms = [(ch, k) for ch in range(2) for k in range(NK)]
                for i, (ch, k) in enumerate(mms):
                    nc.tensor.matmul(
                        q1[:, :],
                        zt_t[:, ch, k, b, :],
                        xt_t[ch][:, b, DM * W * k : DM * W * k + NMOV],
                        start=(i == 0),
                        stop=(i == len(mms) - 1),
                    )
                eb = epool.tile([GH, HIW], _BF16, name="eb", tag="eb", bufs=8)
                # evacuation doubles as the dj-half prefold: the hi groups
                # sit at psum rows 32:41 (32-aligned operand base), shifted
                # 3 cols to absorb their dj offset; tensor ops may read only
                # one PSUM input, so copy the lo half out first
                nc.scalar.copy(out=eb[:], in_=q1[0:GH, 0:HIW])
                nc.vector.tensor_add(
                    out=eb[:], in0=eb[:], in1=q1[32:41, 3:NMOV]
                )
                # late batches alternate onto sync (idle after the xt stream)
                # so the transpose dispatches don't drain past phase end
                dq = nc.sync if (b >= 10 and b % 2 == 0) else nc.gpsimd
                dq.dma_start(
                    out=t16[b : b + 1, :, :].rearrange("p g m -> p (g m)"),
                    in_=eb[:],
                )

            # fold: 8 shifted adds split Vector/GpSimd by their rates
            def g_src(j):
                dd, dja = j // 3, j % 3
                sh = 22 * dd + dja
                return t16[0:BPC, j, sh : sh + O22]

            NV = 5  # sources folded on vector; rest on gpsimd
            accv = work.tile([BPC, HO * W], _BF16, name="accv")
            accg = work.tile([BPC, O22], _BF16, name="accg")
            avs = accv[:, 0:O22]
            nc.vector.tensor_add(out=avs, in0=g_src(0), in1=g_src(1))
            for g in range(2, NV):
                nc.vector.tensor_add(out=avs, in0=avs, in1=g_src(g))
            nc.gpsimd.tensor_add(out=accg[:], in0=g_src(NV), in1=g_src(NV + 1))
            for g in range(NV + 2, GH):
                nc.gpsimd.tensor_add(out=accg[:], in0=accg[:], in1=g_src(g))
            nc.vector.tensor_add(out=avs, in0=avs, in1=accg[:])
            av = accv

            # final: dense 17x17 = acc (22-wide view) + bsum (dense view)
            outb = work.tile([BPC, HO, WO], _F32, name="outb")
            acc_v = av[:, 0 : HO * W].rearrange("b (i j) -> b i j", j=W)[
                :, :, 0:WO
            ]
            bias_v = bsum[:].rearrange("b (i j) -> b i j", j=WO)
            nc.vector.tensor_add(out=outb[:], in0=acc_v, in1=bias_v)
            nc.sync.dma_start(out=out_d[:], in_=outb[:])

    nc.compile()
    return nc


def prep_inputs(x, z, b):
    """Host-side shard + layout prep. Returns per-core in_maps."""
    xb = np.asarray(x).astype(ml_dtypes.bfloat16)
    zb = np.asarray(z).astype(ml_dtypes.bfloat16)
    # exact: bias contributes to the output only through its channel sum
    bsum_all = (
        np.asarray(b).reshape(OO, B, C).sum(axis=2, dtype=np.float32)
    )  # [289, B]
    in_maps = []
    for core in range(NCORES):
        b0 = core * BPC
        xs = xb[b0 : b0 + BPC].reshape(BPC, P, C)
        xT = np.ascontiguousarray(xs.transpose(2, 0, 1)).reshape(2, 128, BPC, P)
        # zT cols: [0:9] = djH0 groups (dd,dja), [32:41] = djH1, rest zero
        zs = zb[b0 : b0 + BPC].reshape(BPC, NK, DM, 2, DM, C)  # b,k,dd,djH,dja,C
        zs = zs.transpose(5, 1, 0, 3, 2, 4)  # C,k,b,djH,dd,dja
        zh = np.ascontiguousarray(zs).reshape(256, NK, BPC, 2, 9)
        zT = np.zeros((256, NK, BPC, MZ), dtype=ml_dtypes.bfloat16)
        zT[..., 0:9] = zh[..., 0, :]
        zT[..., 32:41] = zh[..., 1, :]
        zT = np.ascontiguousarray(
            zT.reshape(2, 128, NK, BPC, MZ).transpose(0, 2, 1, 3, 4)
        )
        bs = np.ascontiguousarray(bsum_all[:, b0 : b0 + BPC].T).astype(
            ml_dtypes.bfloat16
        )
        in_maps.append({"xt": xT, "zt": zT, "bs": bs})
    return in_maps


_cache = {}


def _ensure_ntff_hook():
    """The axon NTFF profile hook normally lives in antenv.axon_hooks, which
    this image lacks; synthesize it from the boot shim's ctypes wrapper."""
    try:
        from antenv.axon_hooks import get_axon_ntff_profile_hook  # noqa: F401
        return True
    except ImportError:
        pass
    try:
        import sys, types
        from trn_agent_boot.trn_boot import _ntff_profile_via_ctypes

        so = os.environ.get("AXON_PJRT_SO", "/opt/axon/libaxon_pjrt.so")
        hook = _ntff_profile_via_ctypes(so)
        mod = types.ModuleType("antenv.axon_hooks")
        mod.get_axon_ntff_profile_hook = lambda: hook
        mod.set_axon_ntff_profile_hook = lambda h: None
        sys.modules["antenv.axon_hooks"] = mod
        import antenv

        antenv.axon_hooks = mod
        return True
    except Exception:
        return False


def kernel(x, z, b):
    from concourse.bass_utils import run_bass_kernel_spmd

    if "nc" not in _cache:
        _cache["nc"] = build_module()
    nc = _cache["nc"]
    in_maps = prep_inputs(x, z, b)
    trace = bool(int(os.environ.get("KERNEL_TRACE", "0") or 0))
    if trace:
        trace = _ensure_ntff_hook()
    res = run_bass_kernel_spmd(
        nc,
        in_maps,
        core_ids=list(range(NCORES)),
        trace=trace,
    )
    _cache["last_result"] = res
    out = np.concatenate([r["out"].reshape(BPC, HO, WO) for r in res.results], axis=0)
    return out[..., None].astype(np.float32)


# revision 21
# speedup vs baseline: 1.0193x; 1.0193x over previous
"""Trainium2 Bass kernel for nn_CorrelationFilter (SiamFC-style correlation).

Math (per batch pair b):
    out[b, oi, oj] = sum_{di<6, dj<6, c<256} x[b, oi+di, oj+dj, c] * z[b, di, dj, c]
                     + sum_{c<256} bias[0, oi, oj, b*256 + c]
with x: [B,22,22,256], z: [B,6,6,256], bias: [1,17,17,B*256], out: [B,17,17,1].

Strategy: pure data parallelism over batch across 8 NeuronCores (16 batches per
core), no cross-core communication. Host does sharding + layout prep (transpose
to channel-major, cast to bf16) and pre-reduces the bias over its channel axis
(mathematically exact: bias enters the output only via sum_c).

Per-core layouts (DM = 3 di's merged per block, NK = 2 blocks, G = 18 groups;
group order g = djH*9 + dd*3 + dja with dj = 3*djH + dja):
  xT [2,128,16,484]      : xT[ch,c,b,p] = x[b, p//22, p%22, ch*128+c]
  zT [2,NK,128,16,G]     : zT[ch,k,c,b,g] = z[b, 3k+dd, dj, ch*128+c]
  bsum [16,289] f32      : bsum[b,o] = sum_c bias[0, o//17, o%17, b*256+c]

Device, one PSUM bank per batch:
  - 4 matmuls (ch,k): stationary zT[ch,k,:,b,:] (K=128, M=18), moving
    xT[ch][:, b, 66k : 66k+418], accumulating:
      Q[g, m] = group (djH,dd,dja) partial at column shift 22dd + 3djH + dja
  - ScalarE evacuation [18, 418] PSUM -> SBUF with f32->bf16 cast
  - one SBUF->SBUF DMA per batch transposes to t16[b, g, m]
  - fold: the shifted group access is affine (addr = 3765 djH + 1276 dd
    + 419 dja + m), so each dj-half folds in ONE strided tensor_reduce:
    Vector takes djH=0, GpSimd takes djH=1, then merge + bias add.

kernel(**inputs) takes FULL unsharded inputs, returns the full output.
"""

import os
import numpy as np
import ml_dtypes

import concourse.bass as bass
import concourse.mybir as mybir
from concourse import bacc
from concourse.tile import TileContext

B, H, W, C = 128, 22, 22, 256
HZ, WZ = 6, 6
HO, WO = 17, 17
OO = HO * WO               # 289 dense output positions
NCORES = 8
BPC = B // NCORES          # 16 batches per core
P = H * W                  # 484 flattened search positions
O22 = (HO - 1) * W + WO    # 369: output span in 22-wide layout

DM = 3                     # di's merged per matmul block
NK = HZ // DM              # 2 matmul blocks per (ch)
G = DM * WZ                # 18 fold groups per batch
NMOV = O22 + (DM - 1) * W + (WZ - 1)  # 418 moving cols per matmul
GH = G // 2                # 9 groups per dj-half
HIW = NMOV - 3             # 415 cols kept after the dj-half prefold
MZ = 41                    # stationary cols: [djH0 9 | zeros | djH1 9 @ 32]

_BF16 = mybir.dt.bfloat16
_F32 = mybir.dt.float32


def build_module():
    nc = bacc.Bacc()
    xt_d = nc.dram_tensor("xt", [2, 128, BPC, P], _BF16, kind="ExternalInput")
    zt_d = nc.dram_tensor("zt", [2, NK, 128, BPC, MZ], _BF16, kind="ExternalInput")
    bs_d = nc.dram_tensor("bs", [BPC, OO], _BF16, kind="ExternalInput")
    out_d = nc.dram_tensor("out", [BPC, HO, WO], _F32, kind="ExternalOutput")

    with TileContext(nc) as tc:
        with (
            tc.tile_pool(name="const", bufs=1) as cpool,
            tc.tile_pool(name="big", bufs=1) as big,
            tc.tile_pool(name="evac", bufs=6) as epool,
            tc.tile_pool(name="work", bufs=1) as work,
            tc.tile_pool(name="psum", bufs=8, space="PSUM") as psum,
        ):
            # stationary z split across queues: ch0 gates the first matmul,
            # so it rides sync ahead of the xt stream (the scalar queue's
            # ACT_TABLE_LOAD preamble would delay it); ch1+bsum on gpsimd
            zt_t = cpool.tile([128, 2, NK, BPC, MZ], _BF16, name="ztt")
            nc.scalar.dma_start(
                out=zt_t[:, 0], in_=zt_d[0].rearrange("k c b g -> c k b g")
            )
            nc.gpsimd.dma_start(
                out=zt_t[:, 1], in_=zt_d[1].rearrange("k c b g -> c k b g")
            )
            bsum = cpool.tile([BPC, OO], _BF16, name="bsum")
            nc.gpsimd.dma_start(out=bsum[:], in_=bs_d[:])

            xt_t = [
                big.tile([128, BPC, P], _BF16, name=f"xt{ch}", tag=f"xt{ch}")
                for ch in range(2)
            ]
            # t16[b, j, m] = Q_b[j, m] + Q_b[hi j, m+3] (dj-halves prefolded
            # in PSUM by the evacuation add; rows j = dd*3 + dja)
            t16 = big.tile([BPC, GH, HIW], _BF16, name="t16")

            # xt chunk schedule: small first chunks to unblock matmul 0 early
            chunks = {0: 2, 2: 2, 4: 4, 8: 4, 12: 4}

            for b in range(BPC):
                if b in chunks:
                    n = chunks[b]
                    for ch in range(2):
                        nc.sync.dma_start(
                            out=xt_t[ch][:, b : b + n, :],
                            in_=xt_d[ch, :, b : b + n, :],
                        )
                q1 = psum.tile([MZ, NMOV], _F32, name="q1", tag="q1", bufs=8)
                mms = [(ch, k) for ch in range(2) for k in range(NK)]
                for i, (ch, k) in enumerate(mms):
                    nc.tensor.matmul(
                        q1[:, :],
                        zt_t[:, ch, k, b, :],
                        xt_t[ch][:, b, DM * W * k : DM * W * k + NMOV],
                        start=(i == 0),
                        stop=(i == len(mms) - 1),
                    )
                eb = epool.tile([GH, HIW], _BF16, name="eb", tag="eb", bufs=8)
                # evacuation doubles as the dj-half prefold: the hi groups
                # sit at psum rows 32:41 (32-aligned operand base), shifted
                # 3 cols to absorb their dj offset; tensor ops may read only
                # one PSUM input, so copy the lo half out first
                nc.scalar.copy(out=eb[:], in_=q1[0:GH, 0:HIW])
                nc.vector.tensor_add(
                    out=eb[:], in0=eb[:], in1=q1[32:41, 3:NMOV]
                )
                # batches past the last xt chunk alternate onto sync (idle
                # then) so the transpose dispatches don't drain past phase
                # end; earlier ones would head-of-line block the xt stream
                dq = nc.sync if (b >= 13 and b % 2 == 1) else nc.gpsimd
                dq.dma_start(
                    out=t16[b : b + 1, :, :].rearrange("p g m -> p (g m)"),
                    in_=eb[:],
                )

            # fold: 8 shifted adds split Vector/GpSimd by their rates
            def g_src(j):
                dd, dja = j // 3, j % 3
                sh = 22 * dd + dja
                return t16[0:BPC, j, sh : sh + O22]

            NV = 5  # sources folded on vector; rest on gpsimd
            accv = work.tile([BPC, HO * W], _BF16, name="accv")
            accg = work.tile([BPC, O22], _BF16, name="accg")
            avs = accv[:, 0:O22]
            nc.vector.tensor_add(out=avs, in0=g_src(0), in1=g_src(1))
            for g in range(2, NV):
                nc.vector.tensor_add(out=avs, in0=avs, in1=g_src(g))
            nc.gpsimd.tensor_add(out=accg[:], in0=g_src(NV), in1=g_src(NV + 1))
            for g in range(NV + 2, GH):
                nc.gpsimd.tensor_add(out=accg[:], in0=accg[:], in1=g_src(g))
            nc.vector.tensor_add(out=avs, in0=avs, in1=accg[:])
            av = accv

            # final: dense 17x17 = acc (22-wide view) + bsum (dense view)
            outb = work.tile([BPC, HO, WO], _F32, name="outb")
            acc_v = av[:, 0 : HO * W].rearrange("b (i j) -> b i j", j=W)[
                :, :, 0:WO
            ]
            bias_v = bsum[:].rearrange("b (i j) -> b i j", j=WO)
            nc.vector.tensor_add(out=outb[:], in0=acc_v, in1=bias_v)
            nc.sync.dma_start(out=out_d[:], in_=outb[:])

    nc.compile()
    return nc


def prep_inputs(x, z, b):
    """Host-side shard + layout prep. Returns per-core in_maps."""
    xb = np.asarray(x).astype(ml_dtypes.bfloat16)
    zb = np.asarray(z).astype(ml_dtypes.bfloat16)
    # exact: bias contributes to the output only through its channel sum
    bsum_all = (
        np.asarray(b).reshape(OO, B, C).sum(axis=2, dtype=np.float32)
    )  # [289, B]
    in_maps = []
    for core in range(NCORES):
        b0 = core * BPC
        xs = xb[b0 : b0 + BPC].reshape(BPC, P, C)
        xT = np.ascontiguousarray(xs.transpose(2, 0, 1)).reshape(2, 128, BPC, P)
        # zT cols: [0:9] = djH0 groups (dd,dja), [32:41] = djH1, rest zero
        zs = zb[b0 : b0 + BPC].reshape(BPC, NK, DM, 2, DM, C)  # b,k,dd,djH,dja,C
        zs = zs.transpose(5, 1, 0, 3, 2, 4)  # C,k,b,djH,dd,dja
        zh = np.ascontiguousarray(zs).reshape(256, NK, BPC, 2, 9)
        zT = np.zeros((256, NK, BPC, MZ), dtype=ml_dtypes.bfloat16)
        zT[..., 0:9] = zh[..., 0, :]
        zT[..., 32:41] = zh[..., 1, :]
        zT = np.ascontiguousarray(
            zT.reshape(2, 128, NK, BPC, MZ).transpose(0, 2, 1, 3, 4)
        )
        bs = np.ascontiguousarray(bsum_all[:, b0 : b0 + BPC].T).astype(
            ml_dtypes.bfloat16
        )
        in_maps.append({"xt": xT, "zt": zT, "bs": bs})
    return in_maps


_cache = {}


def _ensure_ntff_hook():
    """The axon NTFF profile hook normally lives in antenv.axon_hooks, which
    this image lacks; synthesize it from the boot shim's ctypes wrapper."""
    try:
        from antenv.axon_hooks import get_axon_ntff_profile_hook  # noqa: F401
        return True
    except ImportError:
        pass
    try:
        import sys, types
        from trn_agent_boot.trn_boot import _ntff_profile_via_ctypes

        so = os.environ.get("AXON_PJRT_SO", "/opt/axon/libaxon_pjrt.so")
        hook = _ntff_profile_via_ctypes(so)
        mod = types.ModuleType("antenv.axon_hooks")
        mod.get_axon_ntff_profile_hook = lambda: hook
        mod.set_axon_ntff_profile_hook = lambda h: None
        sys.modules["antenv.axon_hooks"] = mod
        import antenv

        antenv.axon_hooks = mod
        return True
    except Exception:
        return False


def kernel(x, z, b):
    from concourse.bass_utils import run_bass_kernel_spmd

    if "nc" not in _cache:
        _cache["nc"] = build_module()
    nc = _cache["nc"]
    in_maps = prep_inputs(x, z, b)
    trace = bool(int(os.environ.get("KERNEL_TRACE", "0") or 0))
    if trace:
        trace = _ensure_ntff_hook()
    res = run_bass_kernel_spmd(
        nc,
        in_maps,
        core_ids=list(range(NCORES)),
        trace=trace,
    )
    _cache["last_result"] = res
    out = np.concatenate([r["out"].reshape(BPC, HO, WO) for r in res.results], axis=0)
    return out[..., None].astype(np.float32)


# revision 22
# speedup vs baseline: 1.0511x; 1.0312x over previous
"""Trainium2 Bass kernel for nn_CorrelationFilter (SiamFC-style correlation).

Math (per batch pair b):
    out[b, oi, oj] = sum_{di<6, dj<6, c<256} x[b, oi+di, oj+dj, c] * z[b, di, dj, c]
                     + sum_{c<256} bias[0, oi, oj, b*256 + c]
with x: [B,22,22,256], z: [B,6,6,256], bias: [1,17,17,B*256], out: [B,17,17,1].

Strategy: pure data parallelism over batch across 8 NeuronCores (16 batches per
core), no cross-core communication. Host does sharding + layout prep (transpose
to channel-major, cast to bf16) and pre-reduces the bias over its channel axis
(mathematically exact: bias enters the output only via sum_c).

Per-core layouts (DM = 3 di's merged per block, NK = 2 blocks, G = 18 groups;
group order g = djH*9 + dd*3 + dja with dj = 3*djH + dja):
  xT [2,128,16,484]      : xT[ch,c,b,p] = x[b, p//22, p%22, ch*128+c]
  zT [2,NK,128,16,G]     : zT[ch,k,c,b,g] = z[b, 3k+dd, dj, ch*128+c]
  bsum [16,289] f32      : bsum[b,o] = sum_c bias[0, o//17, o%17, b*256+c]

Device, one PSUM bank per batch:
  - 4 matmuls (ch,k): stationary zT[ch,k,:,b,:] (K=128, M=18), moving
    xT[ch][:, b, 66k : 66k+418], accumulating:
      Q[g, m] = group (djH,dd,dja) partial at column shift 22dd + 3djH + dja
  - ScalarE evacuation [18, 418] PSUM -> SBUF with f32->bf16 cast
  - one SBUF->SBUF DMA per batch transposes to t16[b, g, m]
  - fold: the shifted group access is affine (addr = 3765 djH + 1276 dd
    + 419 dja + m), so each dj-half folds in ONE strided tensor_reduce:
    Vector takes djH=0, GpSimd takes djH=1, then merge + bias add.

kernel(**inputs) takes FULL unsharded inputs, returns the full output.
"""

import os
import numpy as np
import ml_dtypes

import concourse.bass as bass
import concourse.mybir as mybir
from concourse import bacc
from concourse.tile import TileContext

B, H, W, C = 128, 22, 22, 256
HZ, WZ = 6, 6
HO, WO = 17, 17
OO = HO * WO               # 289 dense output positions
NCORES = 8
BPC = B // NCORES          # 16 batches per core
P = H * W                  # 484 flattened search positions
O22 = (HO - 1) * W + WO    # 369: output span in 22-wide layout

DM = 3                     # di's merged per matmul block
NK = HZ // DM              # 2 matmul blocks per (ch)
G = DM * WZ                # 18 fold groups per batch
NMOV = O22 + (DM - 1) * W + (WZ - 1)  # 418 moving cols per matmul
GH = G // 2                # 9 groups per dj-half
HIW = NMOV - 3             # 415 cols kept after the dj-half prefold
MZ = 41                    # stationary cols: [djH0 9 | zeros | djH1 9 @ 32]

_BF16 = mybir.dt.bfloat16
_F32 = mybir.dt.float32


def build_module():
    nc = bacc.Bacc()
    xt_d = nc.dram_tensor("xt", [2, 128, BPC, P], _BF16, kind="ExternalInput")
    zt_d = nc.dram_tensor("zt", [2, NK, 128, BPC, MZ], _BF16, kind="ExternalInput")
    bs_d = nc.dram_tensor("bs", [BPC, OO], _BF16, kind="ExternalInput")
    out_d = nc.dram_tensor("out", [BPC, HO, WO], _F32, kind="ExternalOutput")

    with TileContext(nc) as tc:
        with (
            tc.tile_pool(name="const", bufs=1) as cpool,
            tc.tile_pool(name="big", bufs=1) as big,
            tc.tile_pool(name="evac", bufs=6) as epool,
            tc.tile_pool(name="work", bufs=1) as work,
            tc.tile_pool(name="psum", bufs=8, space="PSUM") as psum,
        ):
            # stationary z split across queues: ch0 gates the first matmul,
            # so it rides sync ahead of the xt stream (the scalar queue's
            # ACT_TABLE_LOAD preamble would delay it); ch1+bsum on gpsimd
            zt_t = cpool.tile([128, 2, NK, BPC, MZ], _BF16, name="ztt")
            for k in range(NK):
                nc.sync.dma_start(
                    out=zt_t[:, 0, k], in_=zt_d[0, k].rearrange("c b g -> c b g")
                )
            nc.gpsimd.dma_start(
                out=zt_t[:, 1], in_=zt_d[1].rearrange("k c b g -> c k b g")
            )
            bsum = cpool.tile([BPC, OO], _BF16, name="bsum")
            nc.gpsimd.dma_start(out=bsum[:], in_=bs_d[:])

            xt_t = [
                big.tile([128, BPC, P], _BF16, name=f"xt{ch}", tag=f"xt{ch}")
                for ch in range(2)
            ]
            # t16[b, j, m] = Q_b[j, m] + Q_b[hi j, m+3] (dj-halves prefolded
            # in PSUM by the evacuation add; rows j = dd*3 + dja)
            t16 = big.tile([BPC, GH, HIW], _BF16, name="t16")

            # xt chunk schedule: small first chunks to unblock matmul 0 early
            chunks = {0: 2, 2: 2, 4: 4, 8: 4, 12: 4}

            for b in range(BPC):
                if b in chunks:
                    n = chunks[b]
                    for ch in range(2):
                        nc.sync.dma_start(
                            out=xt_t[ch][:, b : b + n, :],
                            in_=xt_d[ch, :, b : b + n, :],
                        )
                q1 = psum.tile([MZ, NMOV], _F32, name="q1", tag="q1", bufs=8)
                mms = [(ch, k) for k in range(NK) for ch in range(2)]
                for i, (ch, k) in enumerate(mms):
                    nc.tensor.matmul(
                        q1[:, :],
                        zt_t[:, ch, k, b, :],
                        xt_t[ch][:, b, DM * W * k : DM * W * k + NMOV],
                        start=(i == 0),
                        stop=(i == len(mms) - 1),
                    )
                eb = epool.tile([GH, HIW], _BF16, name="eb", tag="eb", bufs=8)
                # evacuation doubles as the dj-half prefold: the hi groups
                # sit at psum rows 32:41 (32-aligned operand base), shifted
                # 3 cols to absorb their dj offset; tensor ops may read only
                # one PSUM input, so copy the lo half out first
                nc.scalar.copy(out=eb[:], in_=q1[0:GH, 0:HIW])
                nc.vector.tensor_add(
                    out=eb[:], in0=eb[:], in1=q1[32:41, 3:NMOV]
                )
                nc.gpsimd.dma_start(
                    out=t16[b : b + 1, :, :].rearrange("p g m -> p (g m)"),
                    in_=eb[:],
                )

            # fold: 8 shifted adds split Vector/GpSimd by their rates
            def g_src(j):
                dd, dja = j // 3, j % 3
                sh = 22 * dd + dja
                return t16[0:BPC, j, sh : sh + O22]

            NV = 5  # sources folded on vector; rest on gpsimd
            accv = work.tile([BPC, HO * W], _BF16, name="accv")
            accg = work.tile([BPC, O22], _BF16, name="accg")
            avs = accv[:, 0:O22]
            nc.vector.tensor_add(out=avs, in0=g_src(0), in1=g_src(1))
            for g in range(2, NV):
                nc.vector.tensor_add(out=avs, in0=avs, in1=g_src(g))
            nc.gpsimd.tensor_add(out=accg[:], in0=g_src(NV), in1=g_src(NV + 1))
            for g in range(NV + 2, GH):
                nc.gpsimd.tensor_add(out=accg[:], in0=accg[:], in1=g_src(g))
            nc.vector.tensor_add(out=avs, in0=avs, in1=accg[:])
            av = accv

            # final: dense 17x17 = acc (22-wide view) + bsum (dense view)
            outb = work.tile([BPC, HO, WO], _F32, name="outb")
            acc_v = av[:, 0 : HO * W].rearrange("b (i j) -> b i j", j=W)[
                :, :, 0:WO
            ]
            bias_v = bsum[:].rearrange("b (i j) -> b i j", j=WO)
            nc.vector.tensor_add(out=outb[:], in0=acc_v, in1=bias_v)
            nc.sync.dma_start(out=out_d[:], in_=outb[:])

    nc.compile()
    return nc


def prep_inputs(x, z, b):
    """Host-side shard + layout prep. Returns per-core in_maps."""
    xb = np.asarray(x).astype(ml_dtypes.bfloat16)
    zb = np.asarray(z).astype(ml_dtypes.bfloat16)
    # exact: bias contributes to the output only through its channel sum
    bsum_all = (
        np.asarray(b).reshape(OO, B, C).sum(axis=2, dtype=np.float32)
    )  # [289, B]
    in_maps = []
    for core in range(NCORES):
        b0 = core * BPC
        xs = xb[b0 : b0 + BPC].reshape(BPC, P, C)
        xT = np.ascontiguousarray(xs.transpose(2, 0, 1)).reshape(2, 128, BPC, P)
        # zT cols: [0:9] = djH0 groups (dd,dja), [32:41] = djH1, rest zero
        zs = zb[b0 : b0 + BPC].reshape(BPC, NK, DM, 2, DM, C)  # b,k,dd,djH,dja,C
        zs = zs.transpose(5, 1, 0, 3, 2, 4)  # C,k,b,djH,dd,dja
        zh = np.ascontiguousarray(zs).reshape(256, NK, BPC, 2, 9)
        zT = np.zeros((256, NK, BPC, MZ), dtype=ml_dtypes.bfloat16)
        zT[..., 0:9] = zh[..., 0, :]
        zT[..., 32:41] = zh[..., 1, :]
        zT = np.ascontiguousarray(
            zT.reshape(2, 128, NK, BPC, MZ).transpose(0, 2, 1, 3, 4)
        )
        bs = np.ascontiguousarray(bsum_all[:, b0 : b0 + BPC].T).astype(
            ml_dtypes.bfloat16
        )
        in_maps.append({"xt": xT, "zt": zT, "bs": bs})
    return in_maps


_cache = {}


def _ensure_ntff_hook():
    """The axon NTFF profile hook normally lives in antenv.axon_hooks, which
    this image lacks; synthesize it from the boot shim's ctypes wrapper."""
    try:
        from antenv.axon_hooks import get_axon_ntff_profile_hook  # noqa: F401
        return True
    except ImportError:
        pass
    try:
        import sys, types
        from trn_agent_boot.trn_boot import _ntff_profile_via_ctypes

        so = os.environ.get("AXON_PJRT_SO", "/opt/axon/libaxon_pjrt.so")
        hook = _ntff_profile_via_ctypes(so)
        mod = types.ModuleType("antenv.axon_hooks")
        mod.get_axon_ntff_profile_hook = lambda: hook
        mod.set_axon_ntff_profile_hook = lambda h: None
        sys.modules["antenv.axon_hooks"] = mod
        import antenv

        antenv.axon_hooks = mod
        return True
    except Exception:
        return False


def kernel(x, z, b):
    from concourse.bass_utils import run_bass_kernel_spmd

    if "nc" not in _cache:
        _cache["nc"] = build_module()
    nc = _cache["nc"]
    in_maps = prep_inputs(x, z, b)
    trace = bool(int(os.environ.get("KERNEL_TRACE", "0") or 0))
    if trace:
        trace = _ensure_ntff_hook()
    res = run_bass_kernel_spmd(
        nc,
        in_maps,
        core_ids=list(range(NCORES)),
        trace=trace,
    )
    _cache["last_result"] = res
    out = np.concatenate([r["out"].reshape(BPC, HO, WO) for r in res.results], axis=0)
    return out[..., None].astype(np.float32)
